# revision 27
# baseline (speedup 1.0000x reference)
"""Two-level additive attention pooling on 8 TRN2 NeuronCores.

Reference computation (G=1024 groups, N=512 set size, IN=256, H=128, O=128):
  x       = tanh(feat @ hq_w.T + hq_b)            [G,N,H]
  w1      = softmax(x @ hk_w.T + hk_b, axis=N)    (hk_b drops: softmax-invariant)
  stacked = sum_n w1 * x                          [G,H]
  y       = tanh(stacked @ mq_w.T + mq_b)         [G,H]
  w2      = softmax(y @ mk_w.T + mk_b, axis=G)    (mk_b drops)
  final   = sum_g w2 * y                          [H]
  out     = final @ out_w.T + out_b               [O]

v2 design (vs the f32-streaming baseline at ~296 us):

* feat is cast to fp8(e4m3) on the HOST and streamed at 16 MB/core
  (~47 us of DMA vs ~240 us for f32).  Host layout [g, p, t, n] puts the
  IN contraction on partitions split into 2 "k-tiles" so the whole
  256-deep contraction runs as ONE DoubleRow fp8 matmul per group
  (4x fewer PE cycles than bf16 chunked).
* softmax weight broadcast (e-row -> 128 partitions) also runs as a
  DoubleRow fp8 matmul: exp writes e/2 to fp8, the k-tile dim of the
  moving operand is a stride-0 view, so ones^T @ [e/2; e/2] = e.
* tanh is fused over group PAIRS ([128,1024] over 2 PSUM banks) and exp
  is batched over 4 groups (score rows replicated on 32-partition bands)
  -> ACT engine ~2.9 us/block is the pacer at ~92 us/core.
* level 2 is FULLY LOCAL: the cross-group softmax is a ratio of sums,
  so each core reduces its own 128 groups to partials [P_r = sum e2*y,
  z2_r = sum e2] and ONE 1KB AllReduce(add) finishes the job.  No
  stacked AllGather, no mid-stream collective choreography.
* stages are software-pipelined with explicit block lags (DMA at i,
  matmul+tanh at i-2, scores+exp at i-3, broadcast+weighted-sum at i-4)
  so no engine queue ever head-blocks on another engine's current-block
  work.
"""

import numpy as np
import ml_dtypes

import concourse.bass as bass
import concourse.bacc as bacc
import concourse.tile as tile
from concourse import mybir
from concourse.bass_utils import run_bass_kernel_spmd

F32 = mybir.dt.float32
BF16 = mybir.dt.bfloat16
FP8 = mybir.dt.float8e4
AF = mybir.ActivationFunctionType
ALU = mybir.AluOpType
MPM = mybir.MatmulPerfMode

N_CORES = 8
G, N, IN_DIM, HID, OUT_DIM = 1024, 512, 256, 128, 128
G_LOC = G // N_CORES          # 128 groups per core
GB = 4                        # groups per block
P = 128
KT = 2                        # k-tiles (IN = KT * 128) for DoubleRow
LN2 = 0.6931471805599453


def build_bass(g_loc: int = G_LOC) -> bacc.Bacc:
    G_LOC = g_loc  # noqa: N806 — local override for sim-sized builds
    n_blocks = G_LOC // GB
    nc = bacc.Bacc("TRN2", target_bir_lowering=False, debug=False,
                   num_devices=N_CORES)

    featT = nc.dram_tensor("featT", [G_LOC // GB, P, KT, GB, N], FP8,
                           kind="ExternalInput")
    hq8 = nc.dram_tensor("hq8", [P, KT, HID], FP8, kind="ExternalInput")
    hq_b = nc.dram_tensor("hq_b", [HID, 1], F32, kind="ExternalInput")
    hk32 = nc.dram_tensor("hk32", [P, 32], BF16, kind="ExternalInput")
    mq_wT = nc.dram_tensor("mq_wT", [HID, HID], BF16, kind="ExternalInput")
    mq_b = nc.dram_tensor("mq_b", [HID, 1], F32, kind="ExternalInput")
    mk_w = nc.dram_tensor("mk_w", [HID, 1], BF16, kind="ExternalInput")
    out_wT = nc.dram_tensor("out_wT", [HID, OUT_DIM], F32, kind="ExternalInput")
    out_b = nc.dram_tensor("out_b", [OUT_DIM, 1], F32, kind="ExternalInput")
    zsel = nc.dram_tensor("zsel", [P, GB], BF16, kind="ExternalInput")
    out = nc.dram_tensor("out", [OUT_DIM, 1], F32, kind="ExternalOutput")
    dbg_u = nc.dram_tensor("dbg_u", [P, G_LOC], F32, kind="ExternalOutput")
    dbg_z = nc.dram_tensor("dbg_z", [1, G_LOC], F32, kind="ExternalOutput")

    rg = [list(range(N_CORES))]

    with tile.TileContext(nc) as tc:
        with (
            tc.tile_pool(name="consts", bufs=1) as consts,
            tc.tile_pool(name="accum", bufs=1) as accum,
            tc.tile_pool(name="dram", bufs=1, space="DRAM") as dram,
            tc.tile_pool(name="l2sb", bufs=1) as l2sb,
            tc.tile_pool(name="featp", bufs=4) as featp,
            tc.tile_pool(name="xp", bufs=5) as xp,
            tc.tile_pool(name="ep", bufs=3) as ep,
            tc.tile_pool(name="zp", bufs=3) as zp,
            tc.tile_pool(name="ebcp", bufs=8) as ebcp,
            tc.tile_pool(name="scratchp", bufs=2) as scratchp,
            tc.tile_pool(name="ps_x", bufs=2, space="PSUM") as ps_x,   # 2 banks/tile
            tc.tile_pool(name="ps_s", bufs=1, space="PSUM") as ps_s,
            tc.tile_pool(name="ps_e", bufs=1, space="PSUM") as ps_e,
        ):
            # ---- constants ------------------------------------------------
            hq8_sb = consts.tile([P, KT, HID], FP8)
            nc.sync.dma_start(out=hq8_sb, in_=hq8[:, :, :])
            hk32_sb = consts.tile([P, 32], BF16)
            nc.sync.dma_start(out=hk32_sb, in_=hk32[:, :])
            mq_wT_sb = consts.tile([P, HID], BF16)
            nc.sync.dma_start(out=mq_wT_sb, in_=mq_wT[:, :])
            mk_w_sb = consts.tile([P, 1], BF16)
            nc.sync.dma_start(out=mk_w_sb, in_=mk_w[:, :])
            hq_b_sb = consts.tile([P, 1], F32)
            nc.sync.dma_start(out=hq_b_sb, in_=hq_b[:, :])
            mq_b_sb = consts.tile([P, 1], F32)
            nc.sync.dma_start(out=mq_b_sb, in_=mq_b[:, :])
            out_wT_sb = consts.tile([P, OUT_DIM], F32)
            nc.sync.dma_start(out=out_wT_sb, in_=out_wT[:, :])
            out_b_sb = consts.tile([P, 1], F32)
            nc.sync.dma_start(out=out_b_sb, in_=out_b[:, :])
            zsel_sb = consts.tile([P, GB], BF16)
            nc.sync.dma_start(out=zsel_sb, in_=zsel[:, :])
            # bf16 all-ones, sliceable at any 32-row band (base partition of
            # lhsT must match the e4 slice's)
            ones_bf = consts.tile([P, P], BF16)
            nc.vector.memset(ones_bf, 1.0)
            ones_f32 = consts.tile([1, P], F32)
            nc.vector.memset(ones_f32, 1.0)

            uT_sb = accum.tile([P, G_LOC], F32)        # unnormalized stackedT
            z_row = accum.tile([1, G_LOC], F32)        # per-group softmax denom

            # Warmup collective: boots + caches the CC-core machinery while
            # the feat stream runs, so the real AllReduce at the tail starts
            # hot (the cold path measured ~11 us of pre-mesh startup).
            warm_sb = l2sb.tile([1, 1], F32)
            nc.vector.memset(warm_sb, 0.0)
            warm_in = dram.tile([1, 1], F32, name="warm_in", tag="warm_in")
            warm_out = dram.tile([1, 1], F32, addr_space="Shared",
                                 name="warm_out", tag="warm_out")
            nc.sync.dma_start(out=warm_in[:, :], in_=warm_sb)
            nc.gpsimd.collective_compute(
                "AllReduce", ALU.add, replica_groups=rg,
                ins=[warm_in[:, :].opt()], outs=[warm_out[:, :].opt()])

            # ---- level 1, software-pipelined ------------------------------
            def s_dma(b):
                fb = featp.tile([P, KT, GB, N], FP8, tag="fb")
                nc.sync.dma_start(out=fb, in_=featT[b, :, :, :, :])
                return fb

            fbs = {}
            xt8s = {}
            e4s = {}
            zc4s = {}

            def s_xmm_tanh(b):
                """PE: one DoubleRow fp8 matmul per group PAIR (the k-tile
                dim carries the IN=256 contraction); ACT: tanh per pair over
                2 PSUM banks."""
                fb = fbs.pop(b)
                xt8 = xp.tile([P, GB, N], BF16, tag="xt8")
                for half in range(GB // 2):
                    xps2 = ps_x.tile([P, 2, N], F32, tag="xps2")
                    for i in range(2):
                        nc.tensor.matmul(xps2[:, i, :], hq8_sb[:, :, :],
                                         fb[:, :, 2 * half + i, :],
                                         start=True, stop=True,
                                         perf_mode=MPM.DoubleRow)
                    nc.scalar.activation(xt8[:, 2 * half:2 * half + 2, :],
                                         xps2, AF.Tanh, bias=hq_b_sb)
                xt8s[b] = xt8

            def s_scores(b):
                """PE: scores for 4 groups onto 32-partition bands of one
                PSUM tile; ACT: one exp + free denominators."""
                xt8 = xt8s[b]
                sc4 = ps_s.tile([P, N], F32, tag="sc4")
                for j in range(GB):
                    nc.tensor.matmul(sc4[32 * j:32 * (j + 1), :], hk32_sb,
                                     xt8[:, j, :], start=True, stop=True,
                                     tile_position=(0, 32 * j))
                e4 = ep.tile([P, N], BF16, tag="e4")
                zc4 = zp.tile([P, 1], F32, tag="zc4")
                nc.scalar.activation(e4, sc4, AF.Exp, accum_out=zc4[:, 0:1])
                e4s[b] = e4
                zc4s[b] = zc4

            ebcs = {}

            def s_ebc(b):
                """PE: z flip; DMA: per-group e-row broadcast to all 128
                partitions in SBUF bf16 (stride-0 free-dim source view) —
                split across the HWDGE and SWDGE queues."""
                e4 = e4s.pop(b)
                zc4 = zc4s.pop(b)
                zc4b = zp.tile([P, 1], BF16, tag="zc4b")
                nc.vector.tensor_copy(zc4b, zc4)
                zt4 = ps_e.tile([1, GB], F32, tag="zt4")
                nc.tensor.matmul(zt4, zc4b[:, 0:1], zsel_sb,
                                 start=True, stop=True)
                g0 = b * GB
                nc.vector.tensor_copy(z_row[0:1, g0:g0 + GB], zt4[0:1, :])
                ebcs[b] = []
                for j in range(GB):
                    ebc = ebcp.tile([P, N], BF16, tag="ebc_sb")
                    src = e4[32 * j:32 * j + 1, :].unsqueeze(1).broadcast_to(
                        (1, P, N))
                    eng = nc.sync if j < 2 else nc.gpsimd
                    eng.dma_start(out=ebc, in_=src)
                    ebcs[b].append(ebc)

            def s_stt(b):
                """DVE: weighted sums into stackedT columns — all operands
                SBUF bf16 so the 2x DVE mode applies."""
                g0 = b * GB
                for j in range(GB):
                    prod = scratchp.tile([P, N], BF16, tag="prod")
                    nc.vector.scalar_tensor_tensor(
                        out=prod, in0=xt8s[b][:, j, :], scalar=1.0,
                        in1=ebcs[b][j], op0=ALU.mult, op1=ALU.mult,
                        accum_out=uT_sb[:, g0 + j:g0 + j + 1])
                del xt8s[b]
                del ebcs[b]

            for i in range(n_blocks + 5):
                if i < n_blocks:
                    fbs[i] = s_dma(i)
                if 0 <= i - 2 < n_blocks:
                    s_xmm_tanh(i - 2)
                if 0 <= i - 3 < n_blocks:
                    s_scores(i - 3)
                if 0 <= i - 4 < n_blocks:
                    s_ebc(i - 4)
                if 0 <= i - 5 < n_blocks:
                    s_stt(i - 5)

            # ---- level 2: fully local, then one tiny AllReduce ------------
            nc.sync.dma_start(out=dbg_u[:, :], in_=uT_sb)
            nc.sync.dma_start(out=dbg_z[:, :], in_=z_row)
            inv_z = l2sb.tile([1, G_LOC], F32)
            nc.vector.reciprocal(inv_z, z_row)
            izbc = ps_e.tile([P, G_LOC], F32, tag="ebc", name="izbc")
            nc.tensor.matmul(izbc, ones_f32[0:1, :], inv_z,
                             start=True, stop=True)
            stn = l2sb.tile([P, G_LOC], BF16)
            nc.vector.tensor_mul(stn, uT_sb, izbc)

            yps = ps_x.tile([P, G_LOC], F32, tag="xps2", name="yps")
            nc.tensor.matmul(yps, mq_wT_sb, stn, start=True, stop=True)
            yt = l2sb.tile([P, G_LOC], BF16)
            nc.scalar.activation(yt, yps, AF.Tanh, bias=mq_b_sb)
            s2ps = ps_e.tile([1, G_LOC], F32, tag="ebc", name="s2ps")
            nc.tensor.matmul(s2ps, mk_w_sb, yt, start=True, stop=True)
            e2 = l2sb.tile([1, G_LOC], BF16)
            z2loc = l2sb.tile([1, 1], F32)
            nc.scalar.activation(e2, s2ps, AF.Exp, accum_out=z2loc[0:1, 0:1])
            e2bc = ps_e.tile([P, G_LOC], F32, tag="ebc", name="e2bc")
            nc.tensor.matmul(e2bc, ones_bf[0:1, :], e2,
                             start=True, stop=True)
            scr2 = l2sb.tile([P, G_LOC], BF16)
            pcol = l2sb.tile([P, 1], F32)
            nc.vector.scalar_tensor_tensor(
                out=scr2, in0=yt, scalar=1.0, in1=e2bc,
                op0=ALU.mult, op1=ALU.mult, accum_out=pcol[:, 0:1])

            cc_sb = l2sb.tile([P, 2], F32)
            nc.vector.memset(cc_sb, 0.0)
            nc.vector.tensor_copy(cc_sb[:, 0:1], pcol)
            nc.vector.tensor_copy(cc_sb[0:1, 1:2], z2loc)
            cc_in = dram.tile([P, 2], F32, name="cc_in", tag="cc_in")
            cc_out = dram.tile([P, 2], F32, addr_space="Shared",
                               name="cc_out", tag="cc_out")
            nc.sync.dma_start(out=cc_in[:, :], in_=cc_sb)
            nc.gpsimd.collective_compute(
                "AllReduce", ALU.add, replica_groups=rg,
                ins=[cc_in[:, :].opt()], outs=[cc_out[:, :].opt()])
            red = l2sb.tile([P, 2], F32)
            nc.sync.dma_start(out=red, in_=cc_out[:, :])

            iz2 = l2sb.tile([1, 1], F32)
            nc.vector.reciprocal(iz2, red[0:1, 1:2])
            iz2bc = ps_e.tile([P, 1], F32, tag="ebc", name="iz2bc")
            nc.tensor.matmul(iz2bc, ones_f32[0:1, :], iz2,
                             start=True, stop=True)
            ops = ps_x.tile([P, 1], F32, tag="xps2", name="ops")
            nc.tensor.matmul(ops, out_wT_sb, red[:, 0:1],
                             start=True, stop=True)
            out_sb = l2sb.tile([P, 1], F32)
            nc.vector.scalar_tensor_tensor(
                out=out_sb, in0=ops, scalar=iz2bc[:, 0:1], in1=out_b_sb,
                op0=ALU.mult, op1=ALU.add)
            nc.sync.dma_start(out=out[:, :], in_=out_sb)

    nc.compile()
    return nc


_NC_CACHE = None


def _get_nc():
    global _NC_CACHE
    if _NC_CACHE is None:
        _NC_CACHE = build_bass()
    return _NC_CACHE


def prep_in_maps(inputs: dict) -> list[dict]:
    fp8 = ml_dtypes.float8_e4m3fn
    bf16 = ml_dtypes.bfloat16
    feat = np.asarray(inputs["feat"], dtype=np.float32)
    # [G, N, IN] -> per-core [NB, P, KT, GB, N]: featT[b, p, t, j, n] =
    # feat[b*GB+j, n, t*128 + p] (contraction on partitions, 2 k-tiles,
    # group pairs adjacent in the free dim for paired DoubleRow matmuls)
    nb = G_LOC // GB
    featT = np.ascontiguousarray(
        feat.reshape(N_CORES, nb, GB, N, KT, P).transpose(0, 1, 5, 4, 2, 3)
    ).astype(fp8)

    hq_w = np.asarray(inputs["hq_w"], np.float32)        # [H, IN]
    hq8 = np.ascontiguousarray(
        hq_w.reshape(HID, KT, P).transpose(2, 1, 0)).astype(fp8)

    def col(a, dt=np.float32):
        return np.ascontiguousarray(np.asarray(a, np.float32).reshape(-1, 1)
                                    ).astype(dt)

    hk = np.asarray(inputs["hk_w"], np.float32).reshape(HID, 1)   # [1,H]->[H,1]
    zsel = np.zeros((P, GB), np.float32)
    for j in range(GB):
        zsel[32 * j, j] = 1.0

    shared = {
        "hq8": hq8,
        "hq_b": col(inputs["hq_b"]),
        "hk32": np.ascontiguousarray(np.tile(hk, (1, 32))).astype(bf16),
        "mq_wT": np.ascontiguousarray(
            np.asarray(inputs["mq_w"], np.float32).T).astype(bf16),
        "mq_b": col(inputs["mq_b"]),
        "mk_w": col(inputs["mk_w"], bf16),
        "out_wT": np.ascontiguousarray(np.asarray(inputs["out_w"], np.float32).T),
        "out_b": col(inputs["out_b"]),
        "zsel": zsel.astype(bf16),
    }
    return [{"featT": featT[r], **shared} for r in range(N_CORES)]


def run_sharded(inputs: dict, trace: bool = False, tmpdir: str | None = None):
    """Returns (out [OUT_DIM] np.float32, BassKernelResults)."""
    nc = _get_nc()
    in_maps = prep_in_maps(inputs)
    res = run_bass_kernel_spmd(nc, in_maps, core_ids=list(range(N_CORES)),
                               trace=trace, tmpdir=tmpdir)
    out = np.asarray(res.results[0]["out"], dtype=np.float32).reshape(OUT_DIM)
    return out, res


def kernel(**inputs) -> np.ndarray:
    out, _ = run_sharded(inputs)
    return out


# revision 32
# speedup vs baseline: 1.9312x; 1.9312x over previous
"""Two-level additive attention pooling on 8 TRN2 NeuronCores.

Reference computation (G=1024 groups, N=512 set size, IN=256, H=128, O=128):
  x       = tanh(feat @ hq_w.T + hq_b)            [G,N,H]
  w1      = softmax(x @ hk_w.T + hk_b, axis=N)    (hk_b drops: softmax-invariant)
  stacked = sum_n w1 * x                          [G,H]
  y       = tanh(stacked @ mq_w.T + mq_b)         [G,H]
  w2      = softmax(y @ mk_w.T + mk_b, axis=G)    (mk_b drops)
  final   = sum_g w2 * y                          [H]
  out     = final @ out_w.T + out_b               [O]

v2 design (vs the f32-streaming baseline at ~296 us):

* feat is cast to fp8(e4m3) on the HOST and streamed at 16 MB/core
  (~47 us of DMA vs ~240 us for f32).  Host layout [g, p, t, n] puts the
  IN contraction on partitions split into 2 "k-tiles" so the whole
  256-deep contraction runs as ONE DoubleRow fp8 matmul per group
  (4x fewer PE cycles than bf16 chunked).
* softmax weight broadcast (e-row -> 128 partitions) also runs as a
  DoubleRow fp8 matmul: exp writes e/2 to fp8, the k-tile dim of the
  moving operand is a stride-0 view, so ones^T @ [e/2; e/2] = e.
* tanh is fused over group PAIRS ([128,1024] over 2 PSUM banks) and exp
  is batched over 4 groups (score rows replicated on 32-partition bands)
  -> ACT engine ~2.9 us/block is the pacer at ~92 us/core.
* level 2 is FULLY LOCAL: the cross-group softmax is a ratio of sums,
  so each core reduces its own 128 groups to partials [P_r = sum e2*y,
  z2_r = sum e2] and ONE 1KB AllReduce(add) finishes the job.  No
  stacked AllGather, no mid-stream collective choreography.
* stages are software-pipelined with explicit block lags (DMA at i,
  matmul+tanh at i-2, scores+exp at i-3, broadcast+weighted-sum at i-4)
  so no engine queue ever head-blocks on another engine's current-block
  work.
"""

import numpy as np
import ml_dtypes

import concourse.bass as bass
import concourse.bacc as bacc
import concourse.tile as tile
from concourse import library_config, mybir
from concourse.bass_utils import run_bass_kernel_spmd

F32 = mybir.dt.float32
BF16 = mybir.dt.bfloat16
FP8 = mybir.dt.float8e4
AF = mybir.ActivationFunctionType
ALU = mybir.AluOpType
MPM = mybir.MatmulPerfMode

N_CORES = 8
G, N, IN_DIM, HID, OUT_DIM = 1024, 512, 256, 128, 128
G_LOC = G // N_CORES          # 128 groups per core
GB = 4                        # groups per block
P = 128
KT = 2                        # k-tiles (IN = KT * 128) for DoubleRow
LN2 = 0.6931471805599453


def build_bass(g_loc: int = G_LOC) -> bacc.Bacc:
    G_LOC = g_loc  # noqa: N806 — local override for sim-sized builds
    n_blocks = G_LOC // GB
    nc = bacc.Bacc("TRN2", target_bir_lowering=False, debug=False,
                   num_devices=N_CORES)

    featT = nc.dram_tensor("featT", [G_LOC // GB, P, KT, GB, N], FP8,
                           kind="ExternalInput")
    hq8 = nc.dram_tensor("hq8", [P, KT, HID], FP8, kind="ExternalInput")
    hq_b = nc.dram_tensor("hq_b", [HID, 1], F32, kind="ExternalInput")
    hk32 = nc.dram_tensor("hk32", [P, 32], BF16, kind="ExternalInput")
    mq_wT = nc.dram_tensor("mq_wT", [HID, HID], BF16, kind="ExternalInput")
    mq_b = nc.dram_tensor("mq_b", [HID, 1], F32, kind="ExternalInput")
    mk_w = nc.dram_tensor("mk_w", [HID, 1], BF16, kind="ExternalInput")
    out_wT = nc.dram_tensor("out_wT", [HID, OUT_DIM], F32, kind="ExternalInput")
    out_b = nc.dram_tensor("out_b", [OUT_DIM, 1], F32, kind="ExternalInput")
    zsel = nc.dram_tensor("zsel", [P, GB], BF16, kind="ExternalInput")
    out = nc.dram_tensor("out", [OUT_DIM, 1], F32, kind="ExternalOutput")

    rg = [list(range(N_CORES))]

    with tile.TileContext(nc) as tc:
        with (
            tc.tile_pool(name="consts", bufs=1) as consts,
            tc.tile_pool(name="accum", bufs=1) as accum,
            tc.tile_pool(name="dram", bufs=1, space="DRAM") as dram,
            tc.tile_pool(name="l2sb", bufs=1) as l2sb,
            tc.tile_pool(name="featp", bufs=4) as featp,
            tc.tile_pool(name="xp", bufs=5) as xp,
            tc.tile_pool(name="ep", bufs=3) as ep,
            tc.tile_pool(name="zp", bufs=3) as zp,
            tc.tile_pool(name="ebcp", bufs=8) as ebcp,
            tc.tile_pool(name="scratchp", bufs=2) as scratchp,
            tc.tile_pool(name="ps_x", bufs=2, space="PSUM") as ps_x,   # 2 banks/tile
            tc.tile_pool(name="ps_s", bufs=1, space="PSUM") as ps_s,
            tc.tile_pool(name="ps_e", bufs=2, space="PSUM") as ps_e,
            tc.tile_pool(name="ps_z", bufs=1, space="PSUM") as ps_z,
        ):
            # partition_broadcast (attn gpsimd library) — the HW ucode only
            # honors base partition 0, so it serves exactly the j=0 group.
            nc.gpsimd.load_library(library_config.attn)
            # ---- constants ------------------------------------------------
            hq8_sb = consts.tile([P, KT, HID], FP8)
            nc.sync.dma_start(out=hq8_sb, in_=hq8[:, :, :])
            hk32_sb = consts.tile([P, 32], BF16)
            nc.sync.dma_start(out=hk32_sb, in_=hk32[:, :])
            mq_wT_sb = consts.tile([P, HID], BF16)
            nc.sync.dma_start(out=mq_wT_sb, in_=mq_wT[:, :])
            mk_w_sb = consts.tile([P, 1], BF16)
            nc.sync.dma_start(out=mk_w_sb, in_=mk_w[:, :])
            hq_b_sb = consts.tile([P, 1], F32)
            nc.sync.dma_start(out=hq_b_sb, in_=hq_b[:, :])
            mq_b_sb = consts.tile([P, 1], F32)
            nc.sync.dma_start(out=mq_b_sb, in_=mq_b[:, :])
            out_wT_sb = consts.tile([P, OUT_DIM], F32)
            nc.sync.dma_start(out=out_wT_sb, in_=out_wT[:, :])
            out_b_sb = consts.tile([P, 1], F32)
            nc.sync.dma_start(out=out_b_sb, in_=out_b[:, :])
            zsel_sb = consts.tile([P, GB], BF16)
            nc.sync.dma_start(out=zsel_sb, in_=zsel[:, :])
            # bf16 all-ones, sliceable at any 32-row band (base partition of
            # lhsT must match the e4 slice's)
            ones_bf = consts.tile([P, P], BF16)
            nc.vector.memset(ones_bf, 1.0)
            ones_f32 = consts.tile([1, P], F32)
            nc.vector.memset(ones_f32, 1.0)

            uT_sb = accum.tile([P, G_LOC], F32)        # unnormalized stackedT
            z_row = accum.tile([1, G_LOC], F32)        # per-group softmax denom

            # Warmup collective: boots + caches the CC-core machinery while
            # the feat stream runs, so the real AllReduce at the tail starts
            # hot (the cold path measured ~11 us of pre-mesh startup).
            warm_sb = l2sb.tile([1, 1], F32)
            nc.vector.memset(warm_sb, 0.0)
            warm_in = dram.tile([1, 1], F32, name="warm_in", tag="warm_in")
            warm_out = dram.tile([1, 1], F32, addr_space="Shared",
                                 name="warm_out", tag="warm_out")
            nc.sync.dma_start(out=warm_in[:, :], in_=warm_sb)
            nc.gpsimd.collective_compute(
                "AllReduce", ALU.add, replica_groups=rg,
                ins=[warm_in[:, :].opt()], outs=[warm_out[:, :].opt()])

            # ---- level 1, software-pipelined ------------------------------
            def s_dma(b):
                fb = featp.tile([P, KT, GB, N], FP8, tag="fb")
                nc.sync.dma_start(out=fb, in_=featT[b, :, :, :, :])
                return fb

            fbs = {}
            xt8s = {}
            e4s = {}
            zc4s = {}

            def s_xmm_tanh(b):
                """PE: one DoubleRow fp8 matmul per group PAIR (the k-tile
                dim carries the IN=256 contraction); ACT: tanh per pair over
                2 PSUM banks."""
                fb = fbs.pop(b)
                xt8 = xp.tile([P, GB, N], BF16, tag="xt8")
                for half in range(GB // 2):
                    xps2 = ps_x.tile([P, 2, N], F32, tag="xps2")
                    for i in range(2):
                        nc.tensor.matmul(xps2[:, i, :], hq8_sb[:, :, :],
                                         fb[:, :, 2 * half + i, :],
                                         start=True, stop=True,
                                         perf_mode=MPM.DoubleRow)
                    nc.scalar.activation(xt8[:, 2 * half:2 * half + 2, :],
                                         xps2, AF.Tanh, bias=hq_b_sb)
                xt8s[b] = xt8

            def s_scores(b):
                """PE: scores for 4 groups onto 32-partition bands of one
                PSUM tile; ACT: one exp + free denominators."""
                xt8 = xt8s[b]
                sc4 = ps_s.tile([P, N], F32, tag="sc4")
                for j in range(GB):
                    nc.tensor.matmul(sc4[32 * j:32 * (j + 1), :], hk32_sb,
                                     xt8[:, j, :], start=True, stop=True,
                                     tile_position=(0, 32 * j))
                e4 = ep.tile([P, N], BF16, tag="e4")
                zc4 = zp.tile([P, 1], F32, tag="zc4")
                nc.scalar.activation(e4, sc4, AF.Exp, accum_out=zc4[:, 0:1])
                e4s[b] = e4
                zc4s[b] = zc4

            ebcs = {}

            def s_ebc(b):
                """PE: z flip; DMA: per-group e-row broadcast to all 128
                partitions in SBUF bf16 (stride-0 free-dim source view) —
                split across the HWDGE and SWDGE queues."""
                e4 = e4s.pop(b)
                zc4 = zc4s.pop(b)
                zc4b = zp.tile([P, 1], BF16, tag="zc4b")
                nc.vector.tensor_copy(zc4b, zc4)
                zt4 = ps_z.tile([1, GB], F32, tag="zt4")
                nc.tensor.matmul(zt4, zc4b[:, 0:1], zsel_sb,
                                 start=True, stop=True)
                g0 = b * GB
                nc.vector.tensor_copy(z_row[0:1, g0:g0 + GB], zt4[0:1, :])
                ebcs[b] = []
                # j=0: Pool partition_broadcast (only works from partition 0)
                ebc0 = ebcp.tile([P, N], BF16, tag="ebc_sb")
                nc.gpsimd.partition_broadcast(ebc0[:, :], e4[0:1, :],
                                              channels=P)
                ebcs[b].append(ebc0)
                # j=1,2: SWDGE DMA broadcast (stride-0 free-dim source view);
                # descriptor prep is ~0.6us on the otherwise-idle Pool engine
                for j in (1, 2):
                    ebc = ebcp.tile([P, N], BF16, tag="ebc_sb")
                    nc.gpsimd.dma_start(
                        out=ebc,
                        in_=e4[32 * j:32 * j + 1, :].unsqueeze(1).broadcast_to(
                            (1, P, N)))
                    ebcs[b].append(ebc)
                # j=3: PE ones-broadcast matmul (PSUM f32, 1x stt)
                ebc3 = ps_e.tile([P, N], F32, tag="ebc3")
                nc.tensor.matmul(ebc3, ones_bf[96:97, :], e4[96:97, :],
                                 start=True, stop=True, tile_position=(96, 0))
                ebcs[b].append(ebc3)

            def s_stt(b):
                """DVE: weighted sums into stackedT columns — all operands
                SBUF bf16 so the 2x DVE mode applies."""
                g0 = b * GB
                for j in range(GB):
                    prod = scratchp.tile([P, N], BF16, tag="prod")
                    nc.vector.scalar_tensor_tensor(
                        out=prod, in0=xt8s[b][:, j, :], scalar=1.0,
                        in1=ebcs[b][j], op0=ALU.mult, op1=ALU.mult,
                        accum_out=uT_sb[:, g0 + j:g0 + j + 1])
                del xt8s[b]
                del ebcs[b]

            for i in range(n_blocks + 5):
                if i < n_blocks:
                    fbs[i] = s_dma(i)
                if 0 <= i - 2 < n_blocks:
                    s_xmm_tanh(i - 2)
                if 0 <= i - 3 < n_blocks:
                    s_scores(i - 3)
                if 0 <= i - 4 < n_blocks:
                    s_ebc(i - 4)
                if 0 <= i - 5 < n_blocks:
                    s_stt(i - 5)

            # ---- level 2: fully local, then one tiny AllReduce ------------
            inv_z = l2sb.tile([1, G_LOC], F32)
            nc.vector.reciprocal(inv_z, z_row)
            izbc = ps_e.tile([P, G_LOC], F32, tag="ebc3", name="izbc")
            nc.tensor.matmul(izbc, ones_f32[0:1, :], inv_z,
                             start=True, stop=True)
            stn = l2sb.tile([P, G_LOC], BF16)
            nc.vector.tensor_mul(stn, uT_sb, izbc)

            yps = ps_x.tile([P, G_LOC], F32, tag="xps2", name="yps")
            nc.tensor.matmul(yps, mq_wT_sb, stn, start=True, stop=True)
            yt = l2sb.tile([P, G_LOC], BF16)
            nc.scalar.activation(yt, yps, AF.Tanh, bias=mq_b_sb)
            s2ps = ps_e.tile([1, G_LOC], F32, tag="ebc3", name="s2ps")
            nc.tensor.matmul(s2ps, mk_w_sb, yt, start=True, stop=True)
            e2 = l2sb.tile([1, G_LOC], BF16)
            z2loc = l2sb.tile([1, 1], F32)
            nc.scalar.activation(e2, s2ps, AF.Exp, accum_out=z2loc[0:1, 0:1])
            e2bc = ps_e.tile([P, G_LOC], F32, tag="ebc3", name="e2bc")
            nc.tensor.matmul(e2bc, ones_bf[0:1, :], e2,
                             start=True, stop=True)
            scr2 = l2sb.tile([P, G_LOC], BF16)
            pcol = l2sb.tile([P, 1], F32)
            nc.vector.scalar_tensor_tensor(
                out=scr2, in0=yt, scalar=1.0, in1=e2bc,
                op0=ALU.mult, op1=ALU.mult, accum_out=pcol[:, 0:1])

            cc_sb = l2sb.tile([P, 2], F32)
            nc.vector.memset(cc_sb, 0.0)
            nc.vector.tensor_copy(cc_sb[:, 0:1], pcol)
            nc.vector.tensor_copy(cc_sb[0:1, 1:2], z2loc)
            cc_in = dram.tile([P, 2], F32, name="cc_in", tag="cc_in")
            cc_out = dram.tile([P, 2], F32, addr_space="Shared",
                               name="cc_out", tag="cc_out")
            nc.sync.dma_start(out=cc_in[:, :], in_=cc_sb)
            nc.gpsimd.collective_compute(
                "AllReduce", ALU.add, replica_groups=rg,
                ins=[cc_in[:, :].opt()], outs=[cc_out[:, :].opt()])
            red = l2sb.tile([P, 2], F32)
            nc.sync.dma_start(out=red, in_=cc_out[:, :])

            iz2 = l2sb.tile([1, 1], F32)
            nc.vector.reciprocal(iz2, red[0:1, 1:2])
            iz2bc = ps_e.tile([P, 1], F32, tag="ebc3", name="iz2bc")
            nc.tensor.matmul(iz2bc, ones_f32[0:1, :], iz2,
                             start=True, stop=True)
            ops = ps_x.tile([P, 1], F32, tag="xps2", name="ops")
            nc.tensor.matmul(ops, out_wT_sb, red[:, 0:1],
                             start=True, stop=True)
            out_sb = l2sb.tile([P, 1], F32)
            nc.vector.scalar_tensor_tensor(
                out=out_sb, in0=ops, scalar=iz2bc[:, 0:1], in1=out_b_sb,
                op0=ALU.mult, op1=ALU.add)
            nc.sync.dma_start(out=out[:, :], in_=out_sb)

    nc.compile()
    return nc


_NC_CACHE = None


def _get_nc():
    global _NC_CACHE
    if _NC_CACHE is None:
        _NC_CACHE = build_bass()
    return _NC_CACHE


def prep_in_maps(inputs: dict) -> list[dict]:
    fp8 = ml_dtypes.float8_e4m3fn
    bf16 = ml_dtypes.bfloat16
    feat = np.asarray(inputs["feat"], dtype=np.float32)
    # [G, N, IN] -> per-core [NB, P, KT, GB, N]: featT[b, p, t, j, n] =
    # feat[b*GB+j, n, t*128 + p] (contraction on partitions, 2 k-tiles,
    # group pairs adjacent in the free dim for paired DoubleRow matmuls)
    nb = G_LOC // GB
    featT = np.ascontiguousarray(
        feat.reshape(N_CORES, nb, GB, N, KT, P).transpose(0, 1, 5, 4, 2, 3)
    ).astype(fp8)

    hq_w = np.asarray(inputs["hq_w"], np.float32)        # [H, IN]
    hq8 = np.ascontiguousarray(
        hq_w.reshape(HID, KT, P).transpose(2, 1, 0)).astype(fp8)

    def col(a, dt=np.float32):
        return np.ascontiguousarray(np.asarray(a, np.float32).reshape(-1, 1)
                                    ).astype(dt)

    hk = np.asarray(inputs["hk_w"], np.float32).reshape(HID, 1)   # [1,H]->[H,1]
    zsel = np.zeros((P, GB), np.float32)
    for j in range(GB):
        zsel[32 * j, j] = 1.0

    shared = {
        "hq8": hq8,
        "hq_b": col(inputs["hq_b"]),
        "hk32": np.ascontiguousarray(np.tile(hk, (1, 32))).astype(bf16),
        "mq_wT": np.ascontiguousarray(
            np.asarray(inputs["mq_w"], np.float32).T).astype(bf16),
        "mq_b": col(inputs["mq_b"]),
        "mk_w": col(inputs["mk_w"], bf16),
        "out_wT": np.ascontiguousarray(np.asarray(inputs["out_w"], np.float32).T),
        "out_b": col(inputs["out_b"]),
        "zsel": zsel.astype(bf16),
    }
    return [{"featT": featT[r], **shared} for r in range(N_CORES)]


def run_sharded(inputs: dict, trace: bool = False, tmpdir: str | None = None):
    """Returns (out [OUT_DIM] np.float32, BassKernelResults)."""
    nc = _get_nc()
    in_maps = prep_in_maps(inputs)
    res = run_bass_kernel_spmd(nc, in_maps, core_ids=list(range(N_CORES)),
                               trace=trace, tmpdir=tmpdir)
    out = np.asarray(res.results[0]["out"], dtype=np.float32).reshape(OUT_DIM)
    return out, res


def kernel(**inputs) -> np.ndarray:
    out, _ = run_sharded(inputs)
    return out
